# revision 14
# baseline (speedup 1.0000x reference)
"""Multi-head self-attention (B=2, N=2048, D=1024, H=16) on 8 Trainium2 cores.

Sharding: core c -> batch b = c // 4, head group g = c % 4 (heads 4g..4g+3,
organized as 2 pairs of 2 heads).  Optimized for the Tile cost model:

  * ACT exp stream is the roofline (~133us): 128 exps of [128,1024] paced by a
    double-buffered scores PSUM rotation, emitted at top priority so the
    Activation engine never starves.
  * PV runs d-major (out[i,d]) in bf16: the stationary operand is the exp'd
    scores chunk (full 128-wide), the moving operand is v plus a ones column
    (65 wide) whose output column accumulates the softmax denominator.
    Normalize is a per-partition reciprocal + tensor_scalar_mul; XBAR
    DMA-transpose then builds outT with no PE transpose / PSUM / DVE evac.
    This halves the PE cost of PV vs the transposed-M=65 formulation.
  * All matmul operands are bf16 (inputs cast on host: DMA bytes halved);
    accumulation stays fp32 in PSUM.  proj partials stream out as bf16 via
    gpsimd casting DMA.  Host adds residual and sums 4 partials per batch.
"""

import numpy as np
import ml_dtypes

import concourse.bass as bass
import concourse.bacc as bacc
import concourse.mybir as mybir
import concourse.tile as tile
from concourse.bass_utils import run_bass_kernel_spmd

B = 2
N = 2048
D = 1024
NH = 16
DH = 64
N_CORES = 8
TP = 4                 # head-parallel ways per batch
HPC = NH // TP         # 4 heads per core
PAIRS = 2
HDIM = HPC * DH        # 256 head dims per core
SCALE = 1.0 / 8.0      # 1/sqrt(DH)

IT = N // 512          # 4 i-tiles
JT = N // 128          # 16 j-chunks
KC = D // 128          # 8 feature chunks

F32 = mybir.dt.float32
BF16 = mybir.dt.bfloat16
AF = mybir.ActivationFunctionType


def build_bass():
    nc = bacc.Bacc("TRN2", target_bir_lowering=False, debug=False)
    xq_d = nc.declare_dram_parameter("xq", [128, KC * N], BF16, isOutput=False)
    wq_d = nc.declare_dram_parameter("wq", [128, KC * HDIM], BF16, isOutput=False)
    wk_d = nc.declare_dram_parameter("wk", [128, KC * HDIM], BF16, isOutput=False)
    wv_d = nc.declare_dram_parameter("wv", [128, KC * HDIM], BF16, isOutput=False)
    wp_d = nc.declare_dram_parameter("wp", [128, 2 * D], BF16, isOutput=False)
    o_d = nc.declare_dram_parameter("o", [N, D], BF16, isOutput=True)

    with tile.TileContext(nc) as tc:
        with (
            tc.tile_pool(name="big", bufs=1) as big,
            tc.tile_pool(name="ep", bufs=34) as ep,
            tc.tile_pool(name="sb", bufs=2) as sb,
            tc.tile_pool(name="psum", bufs=1, space="PSUM") as psum,
        ):
            # ---- warm the exp table off the critical path
            wt = big.tile([1, 2], BF16, tag="warm")
            nc.vector.memset(wt[0:1, 0:1], 0.5)
            wo = big.tile([1, 2], BF16, tag="warmo")
            nc.scalar.activation(wo[0:1, 0:1], wt[0:1, 0:1], AF.Exp)
            # ---- PE pstate warm-up: keep the tensor engine busy through the
            # ramp window so the first real matmuls run at full clock.
            dmy = big.tile([1, 256], BF16, tag="dmy")
            nc.vector.memset(dmy[0:1, :], 0.25)
            dacc = psum.tile([128, 260], F32, tag="acc", bufs=2, name="dacc")
            for _ in range(20):
                nc.tensor.matmul(
                    dacc[0:1, 0:256], lhsT=dmy[0:1, 0:1], rhs=dmy[0:1, :],
                    start=True, stop=True,
                )

            # ---- persistent tiles
            xq = big.tile([128, KC * N], BF16, tag="xq")
            wq = big.tile([128, KC * HDIM], BF16, tag="wq")
            wk = big.tile([128, KC * HDIM], BF16, tag="wk")
            wv = big.tile([128, KC * HDIM], BF16, tag="wv")
            wp2 = big.tile([128, 2 * D], BF16, tag="wp")
            qT = [big.tile([128, N], BF16, tag=f"qT{p}", name=f"qT{p}") for p in range(PAIRS)]
            kT = [big.tile([128, N], BF16, tag=f"kT{p}", name=f"kT{p}") for p in range(PAIRS)]
            v_s = big.tile([128, JT * HPC * 65], BF16, tag="v")
            outT2 = big.tile([128, 2 * N], BF16, tag="outT")

            xqr = xq.rearrange("p (c n) -> p c n", c=KC)
            wqr = wq.rearrange("p (q c m) -> p q c m", q=2, c=KC)
            wkr = wk.rearrange("p (q c m) -> p q c m", q=2, c=KC)
            wvr = wv.rearrange("p (c m) -> p c m", c=KC)
            wpr = wp2.rearrange("p (i n) -> p i n", i=2)
            v_sr = v_s.rearrange("p (t h c) -> p t h c", t=JT, h=HPC)
            outTr = outT2.rearrange("p (i n) -> p i n", i=2)

            # ---- input DMAs (sync queue; global DMA serializes in issue order)
            nc.sync.dma_start(out=wk[:, 0:KC * 128], in_=wk_d[:, 0:KC * 128])
            nc.sync.dma_start(out=wq[:, 0:KC * 128], in_=wq_d[:, 0:KC * 128])
            for q in range(4):
                for c in range(KC):
                    base = c * N + q * 512
                    nc.sync.dma_start(
                        out=xq[:, base:base + 512], in_=xq_d[:, base:base + 512]
                    )
                if q == 0:
                    nc.sync.dma_start(out=wv, in_=wv_d[:, :])
                    nc.sync.dma_start(out=wk[:, KC * 128:], in_=wk_d[:, KC * 128:])
                    nc.sync.dma_start(out=wq[:, KC * 128:], in_=wq_d[:, KC * 128:])
            nc.sync.dma_start(out=wp2, in_=wp_d[:, :])
            # denominator ones column (65th col of each v block)
            nc.vector.memset(v_sr[:, :, :, 64:65], 1.0)

            # ---- qkv emitters
            def emit_qk(p, which, it):
                w, dst = (wqr, qT[p]) if which == "q" else (wkr, kT[p])
                ps = psum.tile([128, 512], F32, tag="mm", bufs=2)
                for c in range(KC):
                    nc.tensor.matmul(
                        ps,
                        lhsT=w[:, p, c, :],
                        rhs=xqr[:, c, it * 512:(it + 1) * 512],
                        start=(c == 0),
                        stop=(c == KC - 1),
                    )
                nc.vector.tensor_copy(dst[:, it * 512:(it + 1) * 512], ps)

            def emit_v(t):
                ps = psum.tile([128, 512], F32, tag="mm", bufs=2)
                for c in range(KC):
                    nc.tensor.matmul(
                        ps[:, 0:HDIM],
                        lhsT=xqr[:, c, t * 128:(t + 1) * 128],
                        rhs=wvr[:, c, :],
                        start=(c == 0),
                        stop=(c == KC - 1),
                    )
                nc.vector.tensor_copy(
                    v_sr[:, t, :, 0:64],
                    ps[:, 0:HDIM].rearrange("p (h d) -> p h d", d=64),
                )

            # interleaved kT/qT it0 chains: both ready ~1 chain earlier
            ps_k0 = psum.tile([128, 512], F32, tag="mm", bufs=2)
            ps_q0 = psum.tile([128, 512], F32, tag="mm", bufs=2)
            for c in range(KC):
                for w, ps in ((wkr, ps_k0), (wqr, ps_q0)):
                    nc.tensor.matmul(
                        ps,
                        lhsT=w[:, 0, c, :],
                        rhs=xqr[:, c, 0:512],
                        start=(c == 0),
                        stop=(c == KC - 1),
                    )
            nc.vector.tensor_copy(kT[0][:, 0:512], ps_k0)
            nc.scalar.activation(qT[0][:, 0:512], ps_q0, AF.Copy)

            deferred = []
            deferred += [(0, "qk", 0, "k", 1), (1, "qk", 0, "k", 2), (2, "qk", 0, "k", 3)]
            deferred += [(3, "qk", 0, "q", 1), (4, "qk", 0, "q", 2), (5, "qk", 0, "q", 3)]
            deferred += [(16, "qk", 1, "k", 0), (18, "qk", 1, "k", 1), (20, "qk", 1, "q", 0)]
            deferred += [(22, "qk", 1, "k", 2), (24, "qk", 1, "q", 1), (26, "qk", 1, "k", 3)]
            deferred += [(28, "qk", 1, "q", 2), (30, "qk", 1, "q", 3)]
            deferred.reverse()  # pop() from the end

            b3 = [600_000]

            def emit_b3(fn, *args):
                save = tc.cur_priority
                tc.cur_priority = b3[0]
                fn(*args)
                b3[0] = tc.cur_priority
                tc.cur_priority = save

            def pop_deferred(s):
                while deferred and deferred[-1][0] <= s:
                    _, _, p, which, it = deferred.pop()
                    emit_b3(emit_qk, p, which, it)

            # ---- the exp-paced stream
            for p in range(PAIRS):
                for it in range(IT):
                    accs = None
                    for jt in range(JT):
                        s = (p * IT + it) * JT + jt
                        tc.cur_priority = 100_000 + s * 20
                        # scores^T [j, i] for both heads of the pair (K=64)
                        sct = psum.tile([128, 1024], F32, tag="sc", bufs=2)
                        for h in range(2):
                            nc.tensor.matmul(
                                sct[:, h * 512:(h + 1) * 512],
                                lhsT=kT[p][h * 64:(h + 1) * 64, jt * 128:(jt + 1) * 128],
                                rhs=qT[p][h * 64:(h + 1) * 64, it * 512:(it + 1) * 512],
                                start=True,
                                stop=True,
                            )
                        e1 = ep.tile([128, 1024], BF16, tag="e")
                        nc.scalar.activation(e1, sct, AF.Exp, scale=SCALE)

                        pop_deferred(s)
                        tc.cur_priority = 300_000 + s * 20
                        if p == 0 and it == 0:
                            emit_v(jt)  # must precede the PV that reads it
                        if jt == 0:
                            accs = (
                                psum.tile([128, 260], F32, tag="acc", bufs=2, name="accA"),
                                psum.tile([128, 260], F32, tag="acc", bufs=2, name="accB"),
                            )
                        # PV d-major: stationary e chunk (128 i's), moving v|1
                        for h in range(2):
                            for b in range(4):
                                col = ((b % 2) * 2 + h) * 65
                                nc.tensor.matmul(
                                    accs[b // 2][:, col:col + 65],
                                    lhsT=e1[:, h * 512 + b * 128: h * 512 + (b + 1) * 128],
                                    rhs=v_sr[:, jt, 2 * p + h, :],
                                    start=(jt == 0),
                                    stop=(jt == JT - 1),
                                )

                    # ---- normalize (per-partition recip x 8 groups) ----
                    tc.cur_priority = 300_000 + ((p * IT + it) * JT + JT) * 20 - 10
                    rc = sb.tile([128, 8], F32, tag="rc")
                    for half in range(2):
                        nc.vector.reciprocal(
                            rc[:, half * 4:(half + 1) * 4],
                            accs[half].rearrange("p (g c) -> p g c", c=65)[:, :, 64],
                        )
                    outn = sb.tile([128, 512], BF16, tag="outn")
                    tail = (p == 1 and it == IT - 1)
                    for half in range(2):
                        for g in range(4):
                            b = half * 2 + g // 2
                            h = g % 2
                            odst = outn[:, b * 128 + h * 64: b * 128 + (h + 1) * 64]
                            isrc = accs[half][:, g * 65: g * 65 + 64]
                            sc1 = rc[:, half * 4 + g: half * 4 + g + 1]
                            if tail and g % 2 == 1:
                                nc.scalar.activation(odst, isrc, AF.Copy, scale=sc1)
                            else:
                                nc.vector.tensor_scalar_mul(out=odst, in0=isrc, scalar1=sc1)
                    # XBAR transpose [i,(h d)] -> [(h d), i] straight into outT
                    for b in range(4):
                        nc.sync.dma_start(
                            out=outTr[:, p, it * 512 + b * 128: it * 512 + (b + 1) * 128],
                            in_=outn[:, b * 128:(b + 1) * 128],
                            transpose=True,
                        )

                    # ---- proj for this i-tile once both pairs are in outT
                    if p == 1:
                        tc.cur_priority = b3[0]
                        for bloc in range(4):
                            ic = it * 4 + bloc
                            for ds in range(2):
                                # on the last i-tile alternate psum tags (the
                                # sc rotation is free once exps end) so the
                                # mm+DMA chains of the tail overlap.
                                if it == IT - 1 and (bloc * 2 + ds) % 2 == 1:
                                    pj = psum.tile([128, 1024], F32, tag="sc", bufs=2, name="pjsc")[:, 0:512]
                                else:
                                    pj = psum.tile([128, 512], F32, tag="mm", bufs=2)
                                for pp in range(2):
                                    nc.tensor.matmul(
                                        pj,
                                        lhsT=outTr[:, pp, ic * 128:(ic + 1) * 128],
                                        rhs=wpr[:, pp, ds * 512:(ds + 1) * 512],
                                        start=(pp == 0),
                                        stop=(pp == 1),
                                    )
                                osb = sb.tile([128, 512], BF16, tag="osb", bufs=6)
                                if it == IT - 1 and ds == 1:
                                    nc.scalar.activation(osb, pj, AF.Copy)
                                else:
                                    nc.vector.tensor_copy(osb, pj)
                                nc.sync.dma_start(
                                    out=o_d[ic * 128:(ic + 1) * 128, ds * 512:(ds + 1) * 512],
                                    in_=osb,
                                )
                        b3[0] = tc.cur_priority
    return nc


_NC = None


def _get_nc():
    global _NC
    if _NC is None:
        _NC = build_bass()
        _NC.finalize()
    return _NC


def _chunk_pack(mat):
    """[1024, M] -> [128, 8*M]: feature chunk c at cols c*M..(c+1)*M."""
    M = mat.shape[1]
    return np.ascontiguousarray(
        mat.reshape(KC, 128, M).transpose(1, 0, 2).reshape(128, KC * M)
    )


def _pair_pack(mat):
    """[1024, 256] -> [128, 2*8*128]: pair-major (pair, chunk, 128 cols)."""
    return np.ascontiguousarray(
        mat.reshape(KC, 128, 2, 128).transpose(1, 2, 0, 3).reshape(128, 2 * KC * 128)
    )


def make_in_maps(x, w_qkv, w_proj):
    bf = ml_dtypes.bfloat16
    x = np.asarray(x, np.float32)
    w_qkv = np.asarray(w_qkv, np.float32)
    w_proj = np.asarray(w_proj, np.float32)
    in_maps = []
    for c in range(N_CORES):
        b, g = divmod(c, TP)
        h0 = g * HDIM
        xT = np.ascontiguousarray(x[b].T)
        in_maps.append({
            "xq": _chunk_pack(xT).astype(bf),
            "wq": _pair_pack(w_qkv[:, h0:h0 + HDIM]).astype(bf),
            "wk": _pair_pack(w_qkv[:, D + h0:D + h0 + HDIM]).astype(bf),
            "wv": _chunk_pack(w_qkv[:, 2 * D + h0:2 * D + h0 + HDIM]).astype(bf),
            "wp": np.ascontiguousarray(
                w_proj[h0:h0 + HDIM, :].reshape(2, 128, D).transpose(1, 0, 2).reshape(128, 2 * D)
            ).astype(bf),
        })
    return in_maps


def combine_outputs(x, results):
    x = np.asarray(x, np.float32)
    out = np.empty((B, N, D), np.float32)
    for b in range(B):
        acc = x[b].astype(np.float64)
        for g in range(TP):
            acc += results[b * TP + g]["o"].astype(np.float32)
        out[b] = acc.astype(np.float32)
    return out


def kernel(x, w_qkv, w_proj):
    nc = _get_nc()
    in_maps = make_in_maps(x, w_qkv, w_proj)
    res = run_bass_kernel_spmd(nc, in_maps, list(range(N_CORES))).results
    return combine_outputs(x, res)


# revision 28
# speedup vs baseline: 1.0163x; 1.0163x over previous
"""Multi-head self-attention (B=2, N=2048, D=1024, H=16) on 8 Trainium2 cores.

Sharding: core c -> batch b = c // 4, head group g = c % 4 (heads 4g..4g+3,
organized as 2 pairs of 2 heads).  Each core computes attention for its 4
heads and a per-core projection partial [N, D]; the host adds the residual
and the 4 partials per batch.

Design (driven by the Tile cost model; ~157.5us vs the 217.5us baseline):
  * The Activation-engine exp stream is the roofline: 128 exps of [128,1024]
    (scores for 2 heads x 512 i's) behind a double-buffered scores PSUM
    rotation.  Priority bands (sc/exp < PV/v < qkv/proj) keep the next
    scores matmul ahead of everything else on PE so ACT never starves.
  * PV runs d-major (out[i,d]) in bf16: the stationary operand is a 128-wide
    exp'd-scores chunk (full array), the moving operand is v plus a ones
    column (65 wide) whose output column accumulates the softmax
    denominator.  bf16 keeps 1 cycle/row even at free-dim 65, so this costs
    half the transposed-M=65 formulation.  Normalize is a per-partition
    reciprocal + tensor_scalar_mul; an XBAR DMA-transpose builds outT with
    no PE transpose, no PSUM bank, and no extra evac.
  * All matmul operands are bf16 (host-cast inputs halve DMA bytes, rel err
    ~1e-2 vs the 2e-2 gate); accumulation stays fp32 in PSUM.  PSUM: 4 banks
    scores double-buffer + 2 accumulators + 2 qkv/proj slots = 8.
  * qkv work (q/k tiles, v chunks) is deferred filler popped into the slot
    stream with deadlines; a deep e-tile pool (34) absorbs the PV backlog
    while the qkv chains drain.  Dummy warm-up matmuls hold the PE pstate
    ramp before the first real chain; the last i-tile's proj alternates
    psum tags and evac engines (DVE/ACT) so the tail pipelines into the
    final output DMAs.
"""

import numpy as np
import ml_dtypes

import concourse.bass as bass
import concourse.bacc as bacc
import concourse.mybir as mybir
import concourse.tile as tile
from concourse.bass_utils import run_bass_kernel_spmd

B = 2
N = 2048
D = 1024
NH = 16
DH = 64
N_CORES = 8
TP = 4                 # head-parallel ways per batch
HPC = NH // TP         # 4 heads per core
PAIRS = 2
HDIM = HPC * DH        # 256 head dims per core
SCALE = 1.0 / 8.0      # 1/sqrt(DH)

IT = N // 512          # 4 i-tiles
JT = N // 128          # 16 j-chunks
KC = D // 128          # 8 feature chunks

F32 = mybir.dt.float32
BF16 = mybir.dt.bfloat16
AF = mybir.ActivationFunctionType


def build_bass():
    nc = bacc.Bacc("TRN2", target_bir_lowering=False, debug=False)
    xq_d = nc.declare_dram_parameter("xq", [128, KC * N], BF16, isOutput=False)
    wq_d = nc.declare_dram_parameter("wq", [128, KC * HDIM], BF16, isOutput=False)
    wk_d = nc.declare_dram_parameter("wk", [128, KC * HDIM], BF16, isOutput=False)
    wv_d = nc.declare_dram_parameter("wv", [128, KC * HDIM], BF16, isOutput=False)
    wp_d = nc.declare_dram_parameter("wp", [128, 2 * D], BF16, isOutput=False)
    o_d = nc.declare_dram_parameter("o", [N, D], BF16, isOutput=True)

    with tile.TileContext(nc) as tc:
        with (
            tc.tile_pool(name="big", bufs=1) as big,
            tc.tile_pool(name="ep", bufs=34) as ep,
            tc.tile_pool(name="sb", bufs=2) as sb,
            tc.tile_pool(name="psum", bufs=1, space="PSUM") as psum,
        ):
            # ---- warm the exp table off the critical path
            wt = big.tile([1, 2], BF16, tag="warm")
            nc.vector.memset(wt[0:1, 0:1], 0.5)
            wo = big.tile([1, 2], BF16, tag="warmo")
            nc.scalar.activation(wo[0:1, 0:1], wt[0:1, 0:1], AF.Exp)
            # ---- PE pstate warm-up: keep the tensor engine busy through the
            # ramp window so the first real matmuls run at full clock.
            dmy = big.tile([1, 256], BF16, tag="dmy")
            nc.vector.memset(dmy[0:1, :], 0.25)
            dacc = psum.tile([128, 260], F32, tag="acc", bufs=2, name="dacc")
            for _ in range(8):
                nc.tensor.matmul(
                    dacc[0:1, 0:256], lhsT=dmy[0:1, 0:1], rhs=dmy[0:1, :],
                    start=True, stop=True,
                )

            # ---- persistent tiles
            xq = big.tile([128, KC * N], BF16, tag="xq")
            wq = big.tile([128, KC * HDIM], BF16, tag="wq")
            wk = big.tile([128, KC * HDIM], BF16, tag="wk")
            wv = big.tile([128, KC * HDIM], BF16, tag="wv")
            wp2 = big.tile([128, 2 * D], BF16, tag="wp")
            qT = [big.tile([128, N], BF16, tag=f"qT{p}", name=f"qT{p}") for p in range(PAIRS)]
            kT = [big.tile([128, N], BF16, tag=f"kT{p}", name=f"kT{p}") for p in range(PAIRS)]
            v_s = big.tile([128, JT * HPC * 65], BF16, tag="v")
            outT2 = big.tile([128, 2 * N], BF16, tag="outT")

            xqr = xq.rearrange("p (c n) -> p c n", c=KC)
            wqr = wq.rearrange("p (q c m) -> p q c m", q=2, c=KC)
            wkr = wk.rearrange("p (q c m) -> p q c m", q=2, c=KC)
            wvr = wv.rearrange("p (c m) -> p c m", c=KC)
            wpr = wp2.rearrange("p (i n) -> p i n", i=2)
            v_sr = v_s.rearrange("p (t h c) -> p t h c", t=JT, h=HPC)
            outTr = outT2.rearrange("p (i n) -> p i n", i=2)

            # ---- input DMAs (sync queue; global DMA serializes in issue order)
            nc.sync.dma_start(out=wk[:, 0:KC * 128], in_=wk_d[:, 0:KC * 128])
            nc.sync.dma_start(out=wq[:, 0:KC * 128], in_=wq_d[:, 0:KC * 128])
            for q in range(4):
                for c in range(KC):
                    base = c * N + q * 512
                    nc.sync.dma_start(
                        out=xq[:, base:base + 512], in_=xq_d[:, base:base + 512]
                    )
                if q == 0:
                    nc.sync.dma_start(out=wv, in_=wv_d[:, :])
                if q == 1:
                    nc.sync.dma_start(out=wk[:, KC * 128:], in_=wk_d[:, KC * 128:])
                    nc.sync.dma_start(out=wq[:, KC * 128:], in_=wq_d[:, KC * 128:])
            nc.sync.dma_start(out=wp2, in_=wp_d[:, :])
            # denominator ones column (65th col of each v block)
            nc.vector.memset(v_sr[:, :, :, 64:65], 1.0)

            # ---- qkv emitters
            def emit_qk(p, which, it):
                w, dst = (wqr, qT[p]) if which == "q" else (wkr, kT[p])
                ps = psum.tile([128, 512], F32, tag="mm", bufs=2)
                for c in range(KC):
                    nc.tensor.matmul(
                        ps,
                        lhsT=w[:, p, c, :],
                        rhs=xqr[:, c, it * 512:(it + 1) * 512],
                        start=(c == 0),
                        stop=(c == KC - 1),
                    )
                nc.vector.tensor_copy(dst[:, it * 512:(it + 1) * 512], ps)

            def emit_v(t):
                ps = psum.tile([128, 512], F32, tag="mm", bufs=2)
                for c in range(KC):
                    nc.tensor.matmul(
                        ps[:, 0:HDIM],
                        lhsT=xqr[:, c, t * 128:(t + 1) * 128],
                        rhs=wvr[:, c, :],
                        start=(c == 0),
                        stop=(c == KC - 1),
                    )
                nc.vector.tensor_copy(
                    v_sr[:, t, :, 0:64],
                    ps[:, 0:HDIM].rearrange("p (h d) -> p h d", d=64),
                )

            # interleaved kT/qT it0 chains: both ready ~1 chain earlier
            ps_k0 = psum.tile([128, 512], F32, tag="mm", bufs=2)
            ps_q0 = psum.tile([128, 512], F32, tag="mm", bufs=2)
            for c in range(KC):
                for w, ps in ((wkr, ps_k0), (wqr, ps_q0)):
                    nc.tensor.matmul(
                        ps,
                        lhsT=w[:, 0, c, :],
                        rhs=xqr[:, c, 0:512],
                        start=(c == 0),
                        stop=(c == KC - 1),
                    )
            nc.vector.tensor_copy(kT[0][:, 0:512], ps_k0)
            nc.scalar.activation(qT[0][:, 0:512], ps_q0, AF.Copy)

            deferred = []
            deferred += [(0, "qk", 0, "k", 1), (1, "qk", 0, "k", 2), (2, "qk", 0, "k", 3)]
            deferred += [(3, "qk", 0, "q", 1), (4, "qk", 0, "q", 2), (5, "qk", 0, "q", 3)]
            deferred += [(16, "qk", 1, "k", 0), (18, "qk", 1, "k", 1), (20, "qk", 1, "q", 0)]
            deferred += [(22, "qk", 1, "k", 2), (24, "qk", 1, "q", 1), (26, "qk", 1, "k", 3)]
            deferred += [(28, "qk", 1, "q", 2), (30, "qk", 1, "q", 3)]
            deferred.reverse()  # pop() from the end

            b3 = [600_000]

            def emit_b3(fn, *args):
                save = tc.cur_priority
                tc.cur_priority = b3[0]
                fn(*args)
                b3[0] = tc.cur_priority
                tc.cur_priority = save

            def pop_deferred(s):
                while deferred and deferred[-1][0] <= s:
                    _, _, p, which, it = deferred.pop()
                    emit_b3(emit_qk, p, which, it)

            # ---- the exp-paced stream
            for p in range(PAIRS):
                for it in range(IT):
                    accs = None
                    for jt in range(JT):
                        s = (p * IT + it) * JT + jt
                        tc.cur_priority = 100_000 + s * 20
                        # scores^T [j, i] for both heads of the pair (K=64)
                        sct = psum.tile([128, 1024], F32, tag="sc", bufs=2)
                        for h in range(2):
                            nc.tensor.matmul(
                                sct[:, h * 512:(h + 1) * 512],
                                lhsT=kT[p][h * 64:(h + 1) * 64, jt * 128:(jt + 1) * 128],
                                rhs=qT[p][h * 64:(h + 1) * 64, it * 512:(it + 1) * 512],
                                start=True,
                                stop=True,
                            )
                        e1 = ep.tile([128, 1024], BF16, tag="e")
                        nc.scalar.activation(e1, sct, AF.Exp, scale=SCALE)

                        tc.cur_priority = 300_000 + s * 20
                        if p == 0 and it == 0:
                            emit_v(jt)  # must precede the PV that reads it
                        if jt == 0:
                            accs = (
                                psum.tile([128, 260], F32, tag="acc", bufs=2, name="accA"),
                                psum.tile([128, 260], F32, tag="acc", bufs=2, name="accB"),
                            )
                        # PV d-major: stationary e chunk (128 i's), moving v|1
                        for h in range(2):
                            for b in range(4):
                                col = ((b % 2) * 2 + h) * 65
                                nc.tensor.matmul(
                                    accs[b // 2][:, col:col + 65],
                                    lhsT=e1[:, h * 512 + b * 128: h * 512 + (b + 1) * 128],
                                    rhs=v_sr[:, jt, 2 * p + h, :],
                                    start=(jt == 0),
                                    stop=(jt == JT - 1),
                                )
                        pop_deferred(s)

                    # ---- normalize (per-partition recip x 8 groups) ----
                    tc.cur_priority = 300_000 + ((p * IT + it) * JT + JT) * 20 - 10
                    rc = sb.tile([128, 8], F32, tag="rc")
                    for half in range(2):
                        nc.vector.reciprocal(
                            rc[:, half * 4:(half + 1) * 4],
                            accs[half].rearrange("p (g c) -> p g c", c=65)[:, :, 64],
                        )
                    outn = sb.tile([128, 512], BF16, tag="outn")
                    tail = (p == 1 and it == IT - 1)
                    if p == 1:
                        tc.cur_priority = b3[0]
                    for b in range(4):
                        half = b // 2
                        for h in range(2):
                            g = (b % 2) * 2 + h
                            odst = outn[:, b * 128 + h * 64: b * 128 + (h + 1) * 64]
                            isrc = accs[half][:, g * 65: g * 65 + 64]
                            sc1 = rc[:, half * 4 + g: half * 4 + g + 1]
                            if tail and h == 1:
                                nc.scalar.activation(odst, isrc, AF.Copy, scale=sc1)
                            else:
                                nc.vector.tensor_scalar_mul(out=odst, in0=isrc, scalar1=sc1)
                        # XBAR transpose [i,(h d)] -> [(h d), i] into outT
                        nc.sync.dma_start(
                            out=outTr[:, p, it * 512 + b * 128: it * 512 + (b + 1) * 128],
                            in_=outn[:, b * 128:(b + 1) * 128],
                            transpose=True,
                        )
                        if p == 1:
                            ic = it * 4 + b
                            for ds in range(2):
                                # on the last i-tile alternate psum tags (the
                                # sc rotation is free once exps end) so the
                                # mm+DMA chains of the tail overlap.
                                if it == IT - 1 and (b * 2 + ds) % 2 == 1:
                                    pj = psum.tile([128, 1024], F32, tag="sc", bufs=2, name="pjsc")[:, 0:512]
                                else:
                                    pj = psum.tile([128, 512], F32, tag="mm", bufs=2)
                                for pp in range(2):
                                    nc.tensor.matmul(
                                        pj,
                                        lhsT=outTr[:, pp, ic * 128:(ic + 1) * 128],
                                        rhs=wpr[:, pp, ds * 512:(ds + 1) * 512],
                                        start=(pp == 0),
                                        stop=(pp == 1),
                                    )
                                osb = sb.tile([128, 512], BF16, tag="osb", bufs=6)
                                if it == IT - 1 and ds == 1:
                                    nc.scalar.activation(osb, pj, AF.Copy)
                                else:
                                    nc.vector.tensor_copy(osb, pj)
                                nc.sync.dma_start(
                                    out=o_d[ic * 128:(ic + 1) * 128, ds * 512:(ds + 1) * 512],
                                    in_=osb,
                                )
                    if p == 1:
                        b3[0] = tc.cur_priority
    return nc


_NC = None


def _get_nc():
    global _NC
    if _NC is None:
        _NC = build_bass()
        _NC.finalize()
    return _NC


def _chunk_pack(mat):
    """[1024, M] -> [128, 8*M]: feature chunk c at cols c*M..(c+1)*M."""
    M = mat.shape[1]
    return np.ascontiguousarray(
        mat.reshape(KC, 128, M).transpose(1, 0, 2).reshape(128, KC * M)
    )


def _pair_pack(mat):
    """[1024, 256] -> [128, 2*8*128]: pair-major (pair, chunk, 128 cols)."""
    return np.ascontiguousarray(
        mat.reshape(KC, 128, 2, 128).transpose(1, 2, 0, 3).reshape(128, 2 * KC * 128)
    )


def make_in_maps(x, w_qkv, w_proj):
    bf = ml_dtypes.bfloat16
    x = np.asarray(x, np.float32)
    w_qkv = np.asarray(w_qkv, np.float32)
    w_proj = np.asarray(w_proj, np.float32)
    in_maps = []
    for c in range(N_CORES):
        b, g = divmod(c, TP)
        h0 = g * HDIM
        xT = np.ascontiguousarray(x[b].T)
        in_maps.append({
            "xq": _chunk_pack(xT).astype(bf),
            "wq": _pair_pack(w_qkv[:, h0:h0 + HDIM]).astype(bf),
            "wk": _pair_pack(w_qkv[:, D + h0:D + h0 + HDIM]).astype(bf),
            "wv": _chunk_pack(w_qkv[:, 2 * D + h0:2 * D + h0 + HDIM]).astype(bf),
            "wp": np.ascontiguousarray(
                w_proj[h0:h0 + HDIM, :].reshape(2, 128, D).transpose(1, 0, 2).reshape(128, 2 * D)
            ).astype(bf),
        })
    return in_maps


def combine_outputs(x, results):
    x = np.asarray(x, np.float32)
    out = np.empty((B, N, D), np.float32)
    for b in range(B):
        acc = x[b].astype(np.float64)
        for g in range(TP):
            acc += results[b * TP + g]["o"].astype(np.float32)
        out[b] = acc.astype(np.float32)
    return out


def kernel(x, w_qkv, w_proj):
    nc = _get_nc()
    in_maps = make_in_maps(x, w_qkv, w_proj)
    res = run_bass_kernel_spmd(nc, in_maps, list(range(N_CORES))).results
    return combine_outputs(x, res)


# revision 36
# speedup vs baseline: 1.0223x; 1.0059x over previous
"""Multi-head self-attention (B=2, N=2048, D=1024, H=16) on 8 Trainium2 cores.

Sharding: core c -> batch b = c // 4, head group g = c % 4 (heads 4g..4g+3,
organized as 2 pairs of 2 heads).  Each core computes attention for its 4
heads and a per-core projection partial [N, D]; the host adds the residual
and the 4 partials per batch.

Design (driven by the Tile cost model; ~157.5us vs the 217.5us baseline):
  * The Activation-engine exp stream is the roofline: 128 exps of [128,1024]
    (scores for 2 heads x 512 i's) behind a double-buffered scores PSUM
    rotation.  Priority bands (sc/exp < PV/v < qkv/proj) keep the next
    scores matmul ahead of everything else on PE so ACT never starves.
  * PV runs d-major (out[i,d]) in bf16: the stationary operand is a 128-wide
    exp'd-scores chunk (full array), the moving operand is v plus a ones
    column (65 wide) whose output column accumulates the softmax
    denominator.  bf16 keeps 1 cycle/row even at free-dim 65, so this costs
    half the transposed-M=65 formulation.  Normalize is a per-partition
    reciprocal + tensor_scalar_mul; an XBAR DMA-transpose builds outT with
    no PE transpose, no PSUM bank, and no extra evac.
  * All matmul operands are bf16 (host-cast inputs halve DMA bytes, rel err
    ~1e-2 vs the 2e-2 gate); accumulation stays fp32 in PSUM.  PSUM: 4 banks
    scores double-buffer + 2 accumulators + 2 qkv/proj slots = 8.
  * qkv work (q/k tiles, v chunks) is deferred filler popped into the slot
    stream with deadlines; a deep e-tile pool (34) absorbs the PV backlog
    while the qkv chains drain.  Dummy warm-up matmuls hold the PE pstate
    ramp before the first real chain; the last i-tile's proj alternates
    psum tags and evac engines (DVE/ACT) so the tail pipelines into the
    final output DMAs.
"""

import numpy as np
import ml_dtypes

import concourse.bass as bass
import concourse.bacc as bacc
import concourse.mybir as mybir
import concourse.tile as tile
from concourse.bass_utils import run_bass_kernel_spmd

B = 2
N = 2048
D = 1024
NH = 16
DH = 64
N_CORES = 8
TP = 4                 # head-parallel ways per batch
HPC = NH // TP         # 4 heads per core
PAIRS = 2
HDIM = HPC * DH        # 256 head dims per core
SCALE = 1.0 / 8.0      # 1/sqrt(DH)

IT = N // 512          # 4 i-tiles
JT = N // 128          # 16 j-chunks
KC = D // 128          # 8 feature chunks

F32 = mybir.dt.float32
BF16 = mybir.dt.bfloat16
AF = mybir.ActivationFunctionType


def build_bass():
    nc = bacc.Bacc("TRN2", target_bir_lowering=False, debug=False)
    xq_d = nc.declare_dram_parameter("xq", [128, KC * N], BF16, isOutput=False)
    wq_d = nc.declare_dram_parameter("wq", [128, KC * HDIM], BF16, isOutput=False)
    wk_d = nc.declare_dram_parameter("wk", [128, KC * HDIM], BF16, isOutput=False)
    wv_d = nc.declare_dram_parameter("wv", [128, KC * HDIM], BF16, isOutput=False)
    wp_d = nc.declare_dram_parameter("wp", [128, 2 * D], BF16, isOutput=False)
    o_d = nc.declare_dram_parameter("o", [N, D], BF16, isOutput=True)

    with tile.TileContext(nc) as tc:
        with (
            tc.tile_pool(name="big", bufs=1) as big,
            tc.tile_pool(name="ep", bufs=38) as ep,
            tc.tile_pool(name="sb", bufs=2) as sb,
            tc.tile_pool(name="psum", bufs=1, space="PSUM") as psum,
        ):
            # ---- warm the exp table off the critical path
            wt = big.tile([1, 2], BF16, tag="warm")
            nc.vector.memset(wt[0:1, 0:1], 0.5)
            wo = big.tile([1, 2], BF16, tag="warmo")
            nc.scalar.activation(wo[0:1, 0:1], wt[0:1, 0:1], AF.Exp)
            # ---- PE pstate warm-up: keep the tensor engine busy through the
            # ramp window so the first real matmuls run at full clock.
            dmy = big.tile([1, 256], BF16, tag="dmy")
            nc.vector.memset(dmy[0:1, :], 0.25)
            dacc = psum.tile([128, 260], F32, tag="acc", bufs=2, name="dacc")
            for _ in range(8):
                nc.tensor.matmul(
                    dacc[0:1, 0:256], lhsT=dmy[0:1, 0:1], rhs=dmy[0:1, :],
                    start=True, stop=True,
                )

            # ---- persistent tiles
            xq = big.tile([128, KC * N], BF16, tag="xq")
            wq = big.tile([128, KC * HDIM], BF16, tag="wq")
            wk = big.tile([128, KC * HDIM], BF16, tag="wk")
            wv = big.tile([128, KC * HDIM], BF16, tag="wv")
            wp2 = big.tile([128, 2 * D], BF16, tag="wp")
            qT = [big.tile([128, N], BF16, tag=f"qT{p}", name=f"qT{p}") for p in range(PAIRS)]
            kT = [big.tile([128, N], BF16, tag=f"kT{p}", name=f"kT{p}") for p in range(PAIRS)]
            v_s = big.tile([128, JT * HPC * 65], BF16, tag="v")
            outT2 = big.tile([128, 2 * N], BF16, tag="outT")

            xqr = xq.rearrange("p (c n) -> p c n", c=KC)
            wqr = wq.rearrange("p (q c m) -> p q c m", q=2, c=KC)
            wkr = wk.rearrange("p (q c m) -> p q c m", q=2, c=KC)
            wvr = wv.rearrange("p (c m) -> p c m", c=KC)
            wpr = wp2.rearrange("p (i n) -> p i n", i=2)
            v_sr = v_s.rearrange("p (t h c) -> p t h c", t=JT, h=HPC)
            outTr = outT2.rearrange("p (i n) -> p i n", i=2)

            # ---- input DMAs (sync queue; global DMA serializes in issue order)
            nc.sync.dma_start(out=wk[:, 0:KC * 128], in_=wk_d[:, 0:KC * 128])
            nc.sync.dma_start(out=wq[:, 0:KC * 128], in_=wq_d[:, 0:KC * 128])
            for q in range(4):
                for c in range(KC):
                    base = c * N + q * 512
                    nc.sync.dma_start(
                        out=xq[:, base:base + 512], in_=xq_d[:, base:base + 512]
                    )
                if q == 0:
                    nc.sync.dma_start(out=wv, in_=wv_d[:, :])
                if q == 3:
                    nc.sync.dma_start(out=wk[:, KC * 128:], in_=wk_d[:, KC * 128:])
                    nc.sync.dma_start(out=wq[:, KC * 128:], in_=wq_d[:, KC * 128:])
            nc.sync.dma_start(out=wp2, in_=wp_d[:, :])
            # denominator ones column (65th col of each v block)
            nc.vector.memset(v_sr[:, :, :, 64:65], 1.0)

            # ---- qkv emitters
            def emit_qk(p, which, it):
                w, dst = (wqr, qT[p]) if which == "q" else (wkr, kT[p])
                ps = psum.tile([128, 512], F32, tag="mm", bufs=2)
                for c in range(KC):
                    nc.tensor.matmul(
                        ps,
                        lhsT=w[:, p, c, :],
                        rhs=xqr[:, c, it * 512:(it + 1) * 512],
                        start=(c == 0),
                        stop=(c == KC - 1),
                    )
                nc.vector.tensor_copy(dst[:, it * 512:(it + 1) * 512], ps)

            def emit_v(t):
                ps = psum.tile([128, 512], F32, tag="mm", bufs=2)
                for c in range(KC):
                    nc.tensor.matmul(
                        ps[:, 0:HDIM],
                        lhsT=xqr[:, c, t * 128:(t + 1) * 128],
                        rhs=wvr[:, c, :],
                        start=(c == 0),
                        stop=(c == KC - 1),
                    )
                nc.vector.tensor_copy(
                    v_sr[:, t, :, 0:64],
                    ps[:, 0:HDIM].rearrange("p (h d) -> p h d", d=64),
                )

            # interleaved kT/qT it0 chains: both ready ~1 chain earlier
            ps_k0 = psum.tile([128, 512], F32, tag="mm", bufs=2)
            ps_q0 = psum.tile([128, 512], F32, tag="mm", bufs=2)
            for c in range(KC):
                for w, ps in ((wkr, ps_k0), (wqr, ps_q0)):
                    nc.tensor.matmul(
                        ps,
                        lhsT=w[:, 0, c, :],
                        rhs=xqr[:, c, 0:512],
                        start=(c == 0),
                        stop=(c == KC - 1),
                    )
            nc.vector.tensor_copy(kT[0][:, 0:512], ps_k0)
            nc.scalar.activation(qT[0][:, 0:512], ps_q0, AF.Copy)

            deferred = []
            deferred += [(0, "qk", 0, "k", 1), (1, "qk", 0, "k", 2), (2, "qk", 0, "k", 3)]
            deferred += [(3, "qk", 0, "q", 1), (4, "qk", 0, "q", 2), (5, "qk", 0, "q", 3)]
            deferred += [(16, "qk", 1, "k", 0), (18, "qk", 1, "k", 1), (20, "qk", 1, "q", 0)]
            deferred += [(22, "qk", 1, "k", 2), (24, "qk", 1, "q", 1), (26, "qk", 1, "k", 3)]
            deferred += [(28, "qk", 1, "q", 2), (30, "qk", 1, "q", 3)]
            deferred.reverse()  # pop() from the end

            b3 = [600_000]

            def emit_b3(fn, *args):
                save = tc.cur_priority
                tc.cur_priority = b3[0]
                fn(*args)
                b3[0] = tc.cur_priority
                tc.cur_priority = save

            def pop_deferred(s):
                while deferred and deferred[-1][0] <= s:
                    _, _, p, which, it = deferred.pop()
                    emit_b3(emit_qk, p, which, it)

            # ---- the exp-paced stream
            for p in range(PAIRS):
                for it in range(IT):
                    accs = None
                    for jt in range(JT):
                        s = (p * IT + it) * JT + jt
                        tc.cur_priority = 100_000 + s * 20
                        # scores^T [j, i] for both heads of the pair (K=64)
                        sct = psum.tile([128, 1024], F32, tag="sc", bufs=2)
                        for h in range(2):
                            nc.tensor.matmul(
                                sct[:, h * 512:(h + 1) * 512],
                                lhsT=kT[p][h * 64:(h + 1) * 64, jt * 128:(jt + 1) * 128],
                                rhs=qT[p][h * 64:(h + 1) * 64, it * 512:(it + 1) * 512],
                                start=True,
                                stop=True,
                            )
                        e1 = ep.tile([128, 1024], BF16, tag="e")
                        nc.scalar.activation(e1, sct, AF.Exp, scale=SCALE)

                        tc.cur_priority = 300_000 + s * 20
                        if p == 0 and it == 0:
                            emit_v(jt)  # must precede the PV that reads it
                        if jt == 0:
                            accs = (
                                psum.tile([128, 260], F32, tag="acc", bufs=2, name="accA"),
                                psum.tile([128, 260], F32, tag="acc", bufs=2, name="accB"),
                            )
                        # PV d-major: stationary e chunk (128 i's), moving v|1
                        for h in range(2):
                            for b in range(4):
                                col = ((b % 2) * 2 + h) * 65
                                nc.tensor.matmul(
                                    accs[b // 2][:, col:col + 65],
                                    lhsT=e1[:, h * 512 + b * 128: h * 512 + (b + 1) * 128],
                                    rhs=v_sr[:, jt, 2 * p + h, :],
                                    start=(jt == 0),
                                    stop=(jt == JT - 1),
                                )
                        pop_deferred(s)

                    # ---- normalize (per-partition recip x 8 groups) ----
                    tc.cur_priority = 300_000 + ((p * IT + it) * JT + JT) * 20 - 10
                    rc = sb.tile([128, 8], F32, tag="rc")
                    for half in range(2):
                        nc.vector.reciprocal(
                            rc[:, half * 4:(half + 1) * 4],
                            accs[half].rearrange("p (g c) -> p g c", c=65)[:, :, 64],
                        )
                    outn = sb.tile([128, 512], BF16, tag="outn")
                    tail = (p == 1 and it == IT - 1)
                    if p == 1:
                        tc.cur_priority = b3[0]
                    for b in range(4):
                        half = b // 2
                        for h in range(2):
                            g = (b % 2) * 2 + h
                            odst = outn[:, b * 128 + h * 64: b * 128 + (h + 1) * 64]
                            isrc = accs[half][:, g * 65: g * 65 + 64]
                            sc1 = rc[:, half * 4 + g: half * 4 + g + 1]
                            if tail and h == 1:
                                nc.scalar.activation(odst, isrc, AF.Copy, scale=sc1)
                            else:
                                nc.vector.tensor_scalar_mul(out=odst, in0=isrc, scalar1=sc1)
                        # XBAR transpose [i,(h d)] -> [(h d), i] into outT
                        nc.sync.dma_start(
                            out=outTr[:, p, it * 512 + b * 128: it * 512 + (b + 1) * 128],
                            in_=outn[:, b * 128:(b + 1) * 128],
                            transpose=True,
                        )
                        if p == 1:
                            ic = it * 4 + b
                            for ds in range(2):
                                # on the last i-tile alternate psum tags (the
                                # sc rotation is free once exps end) so the
                                # mm+DMA chains of the tail overlap.
                                if it == IT - 1 and (b * 2 + ds) % 2 == 1:
                                    pj = psum.tile([128, 1024], F32, tag="sc", bufs=2, name="pjsc")[:, 0:512]
                                else:
                                    pj = psum.tile([128, 512], F32, tag="mm", bufs=2)
                                for pp in range(2):
                                    nc.tensor.matmul(
                                        pj,
                                        lhsT=outTr[:, pp, ic * 128:(ic + 1) * 128],
                                        rhs=wpr[:, pp, ds * 512:(ds + 1) * 512],
                                        start=(pp == 0),
                                        stop=(pp == 1),
                                    )
                                osb = sb.tile([128, 512], BF16, tag="osb", bufs=8)
                                if it == IT - 1 and ds == 1:
                                    nc.scalar.activation(osb, pj, AF.Copy)
                                else:
                                    nc.vector.tensor_copy(osb, pj)
                                nc.sync.dma_start(
                                    out=o_d[ic * 128:(ic + 1) * 128, ds * 512:(ds + 1) * 512],
                                    in_=osb,
                                )
                    if p == 1:
                        b3[0] = tc.cur_priority
    return nc


_NC = None


def _get_nc():
    global _NC
    if _NC is None:
        _NC = build_bass()
        _NC.finalize()
    return _NC


def _chunk_pack(mat):
    """[1024, M] -> [128, 8*M]: feature chunk c at cols c*M..(c+1)*M."""
    M = mat.shape[1]
    return np.ascontiguousarray(
        mat.reshape(KC, 128, M).transpose(1, 0, 2).reshape(128, KC * M)
    )


def _pair_pack(mat):
    """[1024, 256] -> [128, 2*8*128]: pair-major (pair, chunk, 128 cols)."""
    return np.ascontiguousarray(
        mat.reshape(KC, 128, 2, 128).transpose(1, 2, 0, 3).reshape(128, 2 * KC * 128)
    )


def make_in_maps(x, w_qkv, w_proj):
    bf = ml_dtypes.bfloat16
    x = np.asarray(x, np.float32)
    w_qkv = np.asarray(w_qkv, np.float32)
    w_proj = np.asarray(w_proj, np.float32)
    in_maps = []
    for c in range(N_CORES):
        b, g = divmod(c, TP)
        h0 = g * HDIM
        xT = np.ascontiguousarray(x[b].T)
        in_maps.append({
            "xq": _chunk_pack(xT).astype(bf),
            "wq": _pair_pack(w_qkv[:, h0:h0 + HDIM]).astype(bf),
            "wk": _pair_pack(w_qkv[:, D + h0:D + h0 + HDIM]).astype(bf),
            "wv": _chunk_pack(w_qkv[:, 2 * D + h0:2 * D + h0 + HDIM]).astype(bf),
            "wp": np.ascontiguousarray(
                w_proj[h0:h0 + HDIM, :].reshape(2, 128, D).transpose(1, 0, 2).reshape(128, 2 * D)
            ).astype(bf),
        })
    return in_maps


def combine_outputs(x, results):
    x = np.asarray(x, np.float32)
    out = np.empty((B, N, D), np.float32)
    for b in range(B):
        acc = x[b].astype(np.float64)
        for g in range(TP):
            acc += results[b * TP + g]["o"].astype(np.float32)
        out[b] = acc.astype(np.float32)
    return out


def kernel(x, w_qkv, w_proj):
    nc = _get_nc()
    in_maps = make_in_maps(x, w_qkv, w_proj)
    res = run_bass_kernel_spmd(nc, in_maps, list(range(N_CORES))).results
    return combine_outputs(x, res)
